# revision 1
# baseline (speedup 1.0000x reference)
"""MoE router kernel for Trainium2 (8 NeuronCores, data-parallel over tokens).

Computes, for h [N, 512]:
    h_proj = h @ W.T                       [N, 64]
    scores = l2norm(h_proj) @ l2norm(E).T  [N, 64]   (cosine)
    full_gates = softmax(scores / tau)
    top2 -> normalized_topk = softmax(top2 values)
    sparse_gates = scatter(normalized_topk at top2 indices)
Returns (sparse_gates [N,64] f32, topk_indices [N,2] i32, full_gates [N,64] f32).

Device strategy (per core, NS = N/8 tokens):
  - Host folds all tiny router params into A = [W.T | (W.T @ e_norm.T)/tau]
    ([512, 128], f64 precision) and pre-transposes h so DMA loads are linear.
  - Per 128-token tile: 4 accumulating fp32 matmuls -> PSUM [128,128] holding
    (h_proj | scores_pre);  scores = scores_pre / ||h_proj||.
  - ACT: Square+accum -> ss; Exp(scale=1/sqrt(ss)) + accum -> exp vals + Z;
    Copy(scale=1/Z) -> full_gates.
  - DVE: max/max_index (top-8 in 2 instrs) -> top-2 values + indices;
    fused tensor_scalar (iota == idx) * gate -> sparse_gates.
  - sigmoid((m1-m2)/Z) on ACT gives the renormalized top-2 weights
    (softmax of 2 == sigmoid of difference).
"""

import numpy as np

N_TOKENS = 262144
IN_DIM = 512
E = 64          # num experts
DE = 64         # router dim
N_CORES = 8
TPB = 128       # tokens per tile (partition dim)
G = 8           # token tiles per supertile
SUP = G * TPB   # tokens per supertile
KC = IN_DIM // 128  # contraction chunks (4)

_CACHE = {}


def _build(ns):
    """Build + compile the per-core Bass program for ns tokens per core."""
    key = ("nc", ns)
    if key in _CACHE:
        return _CACHE[key]

    import concourse.bacc as bacc
    import concourse.tile as tile
    from concourse import mybir

    f32 = mybir.dt.float32
    u32 = mybir.dt.uint32
    i32 = mybir.dt.int32
    AF = mybir.ActivationFunctionType
    OP = mybir.AluOpType

    nsup = ns // SUP
    assert nsup * SUP == ns

    nc = bacc.Bacc("TRN2", target_bir_lowering=False, debug=False)
    hT = nc.dram_tensor("hT", [IN_DIM, ns], f32, kind="ExternalInput").ap()
    A = nc.dram_tensor("A", [IN_DIM, 128], f32, kind="ExternalInput").ap()
    iota = nc.dram_tensor("iota", [TPB, E], f32, kind="ExternalInput").ap()
    sparse = nc.dram_tensor("sparse", [ns, E], f32, kind="ExternalOutput").ap()
    idx_out = nc.dram_tensor("idx", [ns, 2], i32, kind="ExternalOutput").ap()
    fg_out = nc.dram_tensor("fg", [ns, E], f32, kind="ExternalOutput").ap()

    hT_r = hT.rearrange("(c p) n -> p c n", p=128)  # [128, KC, ns]

    with tile.TileContext(nc) as tc:
        with (
            tc.tile_pool(name="const", bufs=1) as const,
            tc.tile_pool(name="hpool", bufs=3) as hpool,
            tc.tile_pool(name="psum", bufs=8, space="PSUM") as psump,
            tc.tile_pool(name="expp", bufs=10) as expp,
            tc.tile_pool(name="fgp", bufs=6) as fgp,
            tc.tile_pool(name="spp", bufs=6) as spp,
            tc.tile_pool(name="stg", bufs=3) as stg,
        ):
            A_sb = const.tile([128, KC, 128], f32)
            nc.sync.dma_start(A_sb, A.rearrange("(c p) m -> p c m", p=128))
            iota_sb = const.tile([TPB, E], f32)
            nc.sync.dma_start(iota_sb, iota)

            for s in range(nsup):
                tok0 = s * SUP
                # ---- load supertile: [128, KC, SUP] (linear DMA) ----
                ht = hpool.tile([128, KC, SUP], f32)
                nc.sync.dma_start(ht, hT_r[:, :, tok0:tok0 + SUP])

                # per-supertile stage tiles
                ss_st = stg.tile([TPB, G], f32, tag="ss")
                inv_st = stg.tile([TPB, G], f32, tag="inv")
                z_st = stg.tile([TPB, G], f32, tag="z")
                rz_st = stg.tile([TPB, G], f32, tag="rz")
                mx_st = stg.tile([TPB, G, 8], f32, tag="mx")
                ix_st = stg.tile([TPB, G, 8], u32, tag="ix")
                diff_st = stg.tile([TPB, G], f32, tag="diff")
                g1_st = stg.tile([TPB, G], f32, tag="g1")
                g2_st = stg.tile([TPB, G], f32, tag="g2")
                i1f_st = stg.tile([TPB, G, 2], f32, tag="i1f")

                psums = []
                for t in range(G):
                    ps = psump.tile([TPB, 128], f32)
                    psums.append(ps)
                    for c in range(KC):
                        nc.tensor.matmul(
                            ps,
                            lhsT=ht[:, c, t * TPB:(t + 1) * TPB],
                            rhs=A_sb[:, c, :],
                            start=(c == 0),
                            stop=(c == KC - 1),
                        )
                    # ss[t] = sum(h_proj^2); squared values discarded in place
                    nc.scalar.activation(
                        out=ps[:, 0:DE], in_=ps[:, 0:DE], func=AF.Square,
                        accum_out=ss_st[:, t:t + 1],
                    )

                # inv = 1/sqrt(ss)  (recip on DVE per accuracy guidance)
                nc.vector.reciprocal(out=inv_st, in_=ss_st)
                nc.scalar.activation(out=inv_st, in_=inv_st, func=AF.Sqrt)

                exps = []
                for t in range(G):
                    ex = expp.tile([TPB, E], f32)
                    exps.append(ex)
                    nc.scalar.activation(
                        out=ex, in_=psums[t][:, DE:128], func=AF.Exp,
                        scale=inv_st[:, t:t + 1], accum_out=z_st[:, t:t + 1],
                    )

                nc.vector.reciprocal(out=rz_st, in_=z_st)

                for t in range(G):
                    r0 = tok0 + t * TPB
                    # full gates = exp * (1/Z)
                    fg_t = fgp.tile([TPB, E], f32)
                    nc.scalar.activation(
                        out=fg_t, in_=exps[t], func=AF.Copy,
                        scale=rz_st[:, t:t + 1],
                    )
                    nc.sync.dma_start(fg_out[r0:r0 + TPB, :], fg_t)
                    # top-8 values + indices
                    nc.vector.max(out=mx_st[:, t, :], in_=exps[t])
                    nc.vector.max_index(
                        out=ix_st[:, t, :], in_max=mx_st[:, t, :], in_values=exps[t],
                    )

                # renormalized top-2 weights: g1 = sigmoid((m1-m2)/Z), g2 = 1-g1
                nc.vector.tensor_tensor(
                    out=diff_st, in0=mx_st[:, :, 0], in1=mx_st[:, :, 1],
                    op=OP.subtract,
                )
                nc.vector.tensor_mul(diff_st, diff_st, rz_st)
                nc.scalar.activation(out=g1_st, in_=diff_st, func=AF.Sigmoid)
                nc.scalar.activation(out=g2_st, in_=diff_st, func=AF.Sigmoid,
                                     scale=-1.0)
                # indices as f32 for the equality masks
                nc.vector.tensor_copy(out=i1f_st, in_=ix_st[:, :, 0:2])

                for t in range(G):
                    r0 = tok0 + t * TPB
                    t1 = spp.tile([TPB, E], f32, tag="t1")
                    nc.vector.tensor_scalar(
                        t1, iota_sb, i1f_st[:, t, 0:1], g1_st[:, t:t + 1],
                        op0=OP.is_equal, op1=OP.mult,
                    )
                    t2 = spp.tile([TPB, E], f32, tag="t2")
                    nc.vector.tensor_scalar(
                        t2, iota_sb, i1f_st[:, t, 1:2], g2_st[:, t:t + 1],
                        op0=OP.is_equal, op1=OP.mult,
                    )
                    sp_t = spp.tile([TPB, E], f32, tag="sp")
                    nc.vector.tensor_add(sp_t, t1, t2)
                    nc.sync.dma_start(sparse[r0:r0 + TPB, :], sp_t)
                    nc.sync.dma_start(
                        idx_out[r0:r0 + TPB, :], ix_st[:, t, 0:2].bitcast(i32),
                    )

    nc.compile()
    _CACHE[key] = nc
    return nc


def _prep_params(W, expert_embeddings, tau):
    e = expert_embeddings.astype(np.float64)
    e_norm = e / np.maximum(np.linalg.norm(e, axis=1, keepdims=True), 1e-12)
    Wd = W.astype(np.float64)
    A = np.concatenate([Wd.T, (Wd.T @ e_norm.T) / float(tau)], axis=1)
    A = np.ascontiguousarray(A, dtype=np.float32)  # [512, 128]
    iota = np.ascontiguousarray(
        np.broadcast_to(np.arange(E, dtype=np.float32), (TPB, E)))
    return A, iota


def kernel(h, W, expert_embeddings, tau):
    from concourse.bass_utils import run_bass_kernel_spmd

    n = h.shape[0]
    ns = n // N_CORES
    A, iota = _prep_params(W, expert_embeddings, tau)
    in_maps = []
    for c in range(N_CORES):
        shard = np.ascontiguousarray(h[c * ns:(c + 1) * ns].T)  # [512, ns]
        in_maps.append({"hT": shard, "A": A, "iota": iota})

    nc = _build(ns)
    res = run_bass_kernel_spmd(nc, in_maps, core_ids=list(range(N_CORES)))
    sparse = np.concatenate([res.results[c]["sparse"] for c in range(N_CORES)])
    idx = np.concatenate([res.results[c]["idx"] for c in range(N_CORES)])
    fg = np.concatenate([res.results[c]["fg"] for c in range(N_CORES)])
    return sparse, idx.astype(np.int32), fg
